# revision 90
# baseline (speedup 1.0000x reference)
"""EnhancedMultiHeadAttention on 8 Trainium2 NeuronCores (Bass/Tile), fp8.

Sharding: core c -> batch b = c//4, head group g = c%4 (4 heads of 16).
Per core, everything is computed in "transposed" layout [feature, token].

v2 (fp8): all matmul operands are float8e4 (e4m3), host-prequantized with
power-of-two scales chosen to keep every stored value in fp8's normal
range (rms ~1..80, max << 240). K>=256 contractions use DoubleRow perf
mode (2 K-tiles per instruction at 0.5 cycles/row = 2x fp8 rate);
attention scores (K=64) stay plain fp8. The LayerNorm mean correction is
a rank-1 fp32r matmul with host-computed column sums of the *quantized*
weights, so quantization enters only through x and the W^T x product.
Scales: x 1.0 | wq,wk 32 | wv,wg,wout 64; descales fold into the exp()
activation scale (scores 1/8192, gate -1/64) and one tensor_scalar in
phase C (1/4096). The AllGather payload is fp8 (0.5 MB/call).
Engine balance: exp/Ln on ACT (single table), x^2 on Pool, row stats and
evacuations on DVE, all partition-broadcasts via stride-0 DMA.
B(qb=0) scores+exp are emitted interleaved with the A-blocks that
produce their k tiles (probs buffered in SBUF, AV deferred) so the ACT
engine - the bottleneck at ~143us busy - starts immediately; phase C of
qb-1 is interleaved at the tail of phase B(qb).
"""

import contextlib
import os

import numpy as np
import ml_dtypes

import jax

jax.config.update("jax_compilation_cache_dir", os.path.expanduser("~/.bass_jax_cache"))
jax.config.update("jax_persistent_cache_min_compile_time_secs", 0.0)
jax.config.update("jax_persistent_cache_min_entry_size_bytes", 0)

import bass_rust
import concourse.bass as bass
import concourse.bacc as bacc
import concourse.tile as tile
from concourse import mybir

_DEP_SYNC = bass_rust.DependencyInfo(sync=True, no_sync=False)
from concourse.bass_utils import run_bass_kernel_spmd
from concourse.hw_specs import get_activation_tables as _orig_gat


def _patched_gat(arch):
    # Steer the greedy ACT-table chooser to the combined ln+exp set so the
    # kernel needs exactly one table load instead of thrashing between
    # exp_and_others and natural_log every block (~2.7us per reload).
    tabs = {k: set(v) for k, v in _orig_gat(arch).items()}
    _AF = mybir.ActivationFunctionType
    for nm in ("exp_and_others", "exp_and_friends"):
        if nm in tabs:
            tabs[nm].discard(_AF.Exp)
    if "natural_log" in tabs:
        tabs["natural_log"].discard(_AF.Ln)
    return tabs


bacc.get_activation_tables = _patched_gat

B, S, D, H, HD = 2, 2048, 1024, 16, 64
NCORES = 8
GROUPS = [[0, 1, 2, 3], [4, 5, 6, 7]]
TB = 512  # token block
NB = S // TB  # 4
DC = D // 128  # 8 K-chunks
NKC = S // 128  # 16 key-position chunks
FH = 4  # heads per core
FQ = FH * HD  # 256 feature columns per core
FP = mybir.dt.float32
FR = mybir.dt.float32r
F16 = mybir.dt.float16
F8 = mybir.dt.float8e4  # e4m3
DRM = mybir.MatmulPerfMode.DoubleRow
AF = mybir.ActivationFunctionType
EPS = 1e-5

SW_QK = 32.0  # q/k weight scale
SW_V = 64.0   # v weight scale (also the scale ctx ships with)
SW_G = 64.0   # gate weight scale
SW_O = 64.0   # out-proj weight scale
SC_EXP = 1.0 / (8.0 * SW_QK * SW_QK)  # scores: 1/sqrt(HD)=1/8 and both SW_QK
SC_OUT = 1.0 / (SW_O * SW_V)

_NC_CACHE = {}


def _bcast_ap(ap, parts):
    if isinstance(ap.ap, (list, tuple)):
        dims = [list(p) for p in ap.ap]
    else:
        dims = [list(p) for p in ap.ap]
    return bass.AP(
        tensor=ap.tensor,
        offset=ap.offset,
        ap=[[0, parts]] + dims,
    )





def _body(tc, t):
    nc = tc.nc
    stack = contextlib.ExitStack()
    stack.enter_context(
        nc.allow_low_precision(reason="fp8/fp16 rounding is intentional; matmul accumulation stays fp32 in PSUM")
    )
    pool = lambda name, bufs, space="SBUF": stack.enter_context(
        tc.tile_pool(name=name, bufs=bufs, space=space)
    )

    consts = pool("consts", 1)
    singles = pool("singles", 1)
    dramp = pool("dramp", 2, "DRAM")

    # PSUM pools (8 banks): sc 2x[128,1024]=4 | ctx 4x[65|128,512]=4.
    # Phases A and C borrow the ctx tags while B-AV is not using them; the
    # B0 scores interleaved with A only touch the sc tags.
    ps_sc = pool("ps_sc", 2, "PSUM")
    ps_ctx = pool("ps_ctx", 1, "PSUM")

    pA_x = pool("pA_x", 4)      # [128, DC, TB] fp8 x block   16KB
    pA_x16 = pool("pA_x16", 3)  # [128, DC, TB] f16 x block   32KB
    pA_sq = pool("pA_sq", 2)    # [128, 2, TB] f16 squares     2KB
    pA_rows = pool("pA_rows", 2)  # [1, TB] msq/var/lnv rows
    pA_ge = pool("pA_ge", 2)    # [128, TB] gate tmp           4KB
    pA_vt = pool("pA_vt", 2)    # [128, FQ] v evac tmp         2KB
    pB_pr0 = pool("pB_pr0", 12)  # qb0 probs, all pairs       32KB
    pB_pr = pool("pB_pr", 8)    # [128, 2, 2TB] fp8 probs     16KB
    pB_rows = pool("pB_rows", 2)  # [4, TB] recip denom
    pB_bcs = pool("pB_bcs", 2)  # [128, TB] denom bcast fp32   8KB
    pB_c2sb = pool("pB_c2sb", 2)  # [128, 2, TB] fp8 norm ctx  2KB
    pB_poev = pool("pB_poev", 1)  # [128, DC, TB] f16 partials 16KB
    pC_rs = pool("pC_rs", 2)    # [128, 2, TB] f16 RS result   4KB
    pC_xr = pool("pC_xr", 2)    # [128, 2, TB] fp32 residual   8KB
    pC_osb = pool("pC_osb", 2)  # [128, 2, TB] fp32 out staging 8KB

    # constants
    # [128, 2, 16] so the DoubleRow weight AP's tile-dim step is 16
    # (s3_lw dual-fp8 restriction: step % 16 == 0)
    ones_col8 = consts.tile([128, 2, 16], F8)
    nc.vector.memset(ones_col8, 1.0)
    ones_col16 = consts.tile([128, 1], F16)
    nc.vector.memset(ones_col16, 1.0)
    onesf_row = consts.tile([1, 128], FP)
    nc.vector.memset(onesf_row, 1.0)
    ones_row = consts.tile([1, 128], FR)
    nc.vector.tensor_copy(out=ones_row, in_=onesf_row)
    eps_t = consts.tile([1, 1], FP)
    nc.vector.memset(eps_t, EPS)


    # resident weights (fp8, host-prequantized; DMAs deferred until after
    # the first x block so the LN stats pipeline starts immediately)
    wqkg_sb = singles.tile([128, DC, 3 * FQ], F8)
    wv_sb = singles.tile([128, DC, FQ], F8)
    woutr_sb = singles.tile([64, FH, D], F8)
    ncs_sb = singles.tile([1, 3 * FQ], FP)
    ncsv_sb = singles.tile([1, FQ], FP)
    ncs_fr = singles.tile([1, 3 * FQ], FR)
    ncsv_fr = singles.tile([1, FQ], FR)


    def load_weights():
        nc.sync.dma_start(out=wqkg_sb, in_=t["wqkg"].ap().rearrange("(d p) f -> p d f", p=128))
        nc.sync.dma_start(out=wv_sb, in_=t["wv"].ap().rearrange("(d p) f -> p d f", p=128))
        nc.sync.dma_start(out=woutr_sb, in_=t["wout"].ap().rearrange("(h p) f -> p h f", p=64))
        nc.sync.dma_start(out=ncs_sb, in_=t["ncs"].ap().rearrange("(o f) -> o f", o=1))
        nc.sync.dma_start(out=ncsv_sb, in_=t["ncsv"].ap().rearrange("(o f) -> o f", o=1))
        # walrus requires true fp32r rounding for fp32r matmul operands
        nc.vector.tensor_copy(out=ncs_fr, in_=ncs_sb)
        nc.vector.tensor_copy(out=ncsv_fr, in_=ncsv_sb)

    # resident activations + per-block LN stats
    qT = singles.tile([128, 2, S], F8)
    kT = singles.tile([128, 2, S], F8)
    gT = singles.tile([128, 2, S], F16)
    # last dim padded to 80 so DoubleRow lhsT kc-step (FH*80=320B) is %16;
    # column 64 is the all-ones denominator column, 65..79 unused
    va = singles.tile([128, NKC, FH, 80], F8)
    nc.vector.memset(va[:, :, :, HD:HD + 1], 1.0)
    pA_mu = pool("pA_mu", 4)    # [1, TB] FR mean rows
    pA_rsb = pool("pA_rsb", 4)  # [128, TB] rstd broadcast fp32
    pA_rsc = pool("pA_rsc", 4)  # [128, 4] rstd columns FR
    mus, rsbs, rscs = {}, {}, {}

    xT_r = t["xT"].ap().rearrange("(d p) tk -> p d tk", p=128)
    xT16_r = t["xT16"].ap().rearrange("(d p) tk -> p d tk", p=128)
    xres_r = t["xres"].ap().rearrange("(m p) tk -> p m tk", p=128)

    xblks = {}

    x16s = {}

    # x loads are emitted for all blocks up front (pools hold all 4) so the
    # small dependent-wait DMAs (rsc, bcd) never block bulk loads in the SP
    # queue.
    def load_x(i):
        tb = slice(i * TB, (i + 1) * TB)
        x16 = pA_x16.tile([128, DC, TB], F16, tag="x16", name=f"x16_{i}")
        nc.sync.dma_start(out=x16, in_=xT16_r[:, :, tb])
        x16s[i] = x16
        xblk = pA_x.tile([128, DC, TB], F8, tag="xblk", name=f"xblk{i}")
        nc.sync.dma_start(out=xblk, in_=xT_r[:, :, tb])
        xblks[i] = xblk

    # ---------------- Phase A0: LN stats for one token block --------------
    def phase_a0(i):
        xblk = xblks[i]
        psx = ps_ctx.tile([1, TB], FP, tag="cx0", name=f"psx{i}")
        for d2 in range(DC // 2):
            nc.tensor.matmul(
                out=psx, lhsT=ones_col8[:, :, 0:1], rhs=xblk[:, 2 * d2:2 * d2 + 2, :],
                start=(d2 == 0), stop=(d2 == DC // 2 - 1), perf_mode=DRM,
            )
        # x^2 on DVE at 2x fp16 rate (all operands 2-byte) from the f16 x copy
        x16 = x16s.pop(i)
        pssq = ps_sc.tile([1, TB], FP, tag="sc", name=f"pssq{i}")
        for d in range(DC):
            xsq = pA_sq.tile([128, TB], F16, tag="xsq", name=f"xsq{i}_{d}")
            nc.vector.tensor_mul(out=xsq, in0=x16[:, d, :], in1=x16[:, d, :])
            nc.tensor.matmul(
                out=pssq, lhsT=ones_col16, rhs=xsq,
                start=(d == 0), stop=(d == DC - 1),
            )
        mu = pA_mu.tile([1, TB], FR, tag="mu", name=f"mu{i}")
        mus[i] = mu
        nc.vector.tensor_scalar_mul(out=mu, in0=psx, scalar1=1.0 / D)
        msq = pA_rows.tile([1, TB], FP, tag="msq", name=f"msq{i}")
        nc.vector.tensor_scalar_mul(out=msq, in0=pssq, scalar1=1.0 / D)
        var = pA_rows.tile([1, TB], FP, tag="var", name=f"var{i}")
        nc.vector.tensor_mul(out=var, in0=mu, in1=mu)
        nc.vector.tensor_sub(out=var, in0=msq, in1=var)
        # rstd = exp(-0.5 * ln(var + eps))  (keeps everything in one ACT table set)
        nc.scalar.activation(out=var, in_=var, func=AF.Ln, bias=eps_t[0:1, :])
        rstd = pA_rows.tile([1, TB], FR, tag="rstd", name=f"rstd{i}")
        nc.scalar.activation(out=rstd, in_=var, func=AF.Exp, scale=-0.5)
        # broadcast rstd to all partitions: K=1 ones matmul, ACT evacuation
        pbc = ps_ctx.tile([128, TB], FP, tag="cx1", name=f"pbcrs{i}")
        nc.tensor.matmul(out=pbc, lhsT=ones_row, rhs=rstd, start=True, stop=True)
        rs_b = pA_rsb.tile([128, TB], FP, tag="rs_b", name=f"rsb{i}")
        rsbs[i] = rs_b
        nc.scalar.activation(out=rs_b, in_=pbc, func=AF.Copy)
        rsc = pA_rsc.tile([128, 4], FR, tag="rsc", name=f"rsc{i}")
        rscs[i] = rsc
        for a in range(4):
            nc.sync.dma_start(
                out=rsc[:, a:a + 1], in_=rstd[0:1, a * 128:(a + 1) * 128]
            )

    # ---------------- Phase A1: projections for one token block -----------
    def phase_a1_qkg(i):
        tb = slice(i * TB, (i + 1) * TB)
        xblk = xblks[i]
        mu = mus[i]
        rs_b = rsbs.pop(i)
        # q/k/gate projections on RAW fp8 x; mean subtraction folded in as a
        # rank-1 fp32r correction (ncs = -colsum(W8)); rstd at evacuation:
        #   W8^T((x-mu)rstd) = rstd * (W8^T x + ncs * mu)
        for m in range(6):
            pqk = ps_ctx.tile([128, TB], FP, tag=f"cx{m % 4}", name=f"pqk{i}_{m}")
            for d2 in range(DC // 2):
                nc.tensor.matmul(
                    out=pqk,
                    lhsT=wqkg_sb[:, 2 * d2:2 * d2 + 2, m * 128:(m + 1) * 128],
                    rhs=xblk[:, 2 * d2:2 * d2 + 2, :],
                    start=(d2 == 0), stop=False, perf_mode=DRM,
                )
            nc.tensor.matmul(
                out=pqk, lhsT=ncs_fr[0:1, m * 128:(m + 1) * 128],
                rhs=mu, start=False, stop=True,
            )
            if m < 4:
                dst = qT[:, m, tb] if m < 2 else kT[:, m - 2, tb]
                nc.vector.tensor_mul(out=dst, in0=pqk, in1=rs_b)
            else:
                # gate = sigmoid(u) = 1 / (1 + exp(-u)); u arrives x SW_G
                ge = pA_ge.tile([128, TB], FP, tag="ge", name=f"ge{i}_{m}")
                nc.vector.tensor_mul(out=ge, in0=pqk, in1=rs_b)
                nc.scalar.activation(out=ge, in_=ge, func=AF.Exp, scale=-1.0 / SW_G)
                nc.gpsimd.tensor_scalar_add(out=ge, in0=ge, scalar1=1.0)
                nc.vector.reciprocal(out=gT[:, m - 4, tb], in_=ge)

    def phase_a1_v(i):
        # v projection on RAW fp8 x (x as stationary side): [tok, feat];
        # mean correction mu (x) ncsv; rstd is per-token at evacuation
        xblk = xblks.pop(i)
        mu = mus.pop(i)
        rsc = rscs.pop(i)
        for mt in range(4):
            kcg = i * 4 + mt
            pv = ps_ctx.tile([128, FH, HD], FP, tag=f"cx{(mt + 2) % 4}", name=f"pv{i}_{mt}")
            for d2 in range(DC // 2):
                nc.tensor.matmul(
                    out=pv,
                    lhsT=xblk[:, 2 * d2:2 * d2 + 2, mt * 128:(mt + 1) * 128],
                    rhs=wv_sb[:, 2 * d2:2 * d2 + 2, :],
                    start=(d2 == 0), stop=False, perf_mode=DRM,
                )
            nc.tensor.matmul(
                out=pv, lhsT=mu[0:1, mt * 128:(mt + 1) * 128],
                rhs=ncsv_fr[0:1, :],
                start=False, stop=True,
            )
            nc.vector.tensor_scalar_mul(
                out=va[:, kcg, :, 0:HD],
                in0=pv, scalar1=rsc[:, mt:mt + 1].bitcast(FP),
            )

    # ------- Phase B (attention) / AG / Phase C (output) ------------------
    def b_scores_pair(qb, pair, pr):
        """scores+exp for kc = 2*pair, 2*pair+1 into pr [128, 2, 2, TB].
        Returns the last scores matmul (an ordering anchor for AV)."""
        qs = slice(qb * TB, (qb + 1) * TB)
        last_mm = None
        for par in range(2):
            kc = 2 * pair + par
            for half in range(2):
                sc = ps_sc.tile([128, 2 * TB], FP, tag="sc",
                                name=f"sc{qb}_{kc}_{half}")
                for j in range(2):
                    last_mm = nc.tensor.matmul(
                        out=sc[:, j * TB:(j + 1) * TB],
                        lhsT=kT[j * 64:(j + 1) * 64, half, kc * 128:(kc + 1) * 128],
                        rhs=qT[j * 64:(j + 1) * 64, half, qs],
                        start=True, stop=True, skip_group_check=True,
                    )
                nc.scalar.activation(
                    out=pr[half][:, par, :], in_=sc, func=AF.Exp, scale=SC_EXP
                )
        return last_mm

    def b_av_pair(pair, pr, st, after=None):
        # `after` pins AV behind a later scores matmul in the PE stream:
        # AV(pair0) waits on the previous qb's b_finish to release the ctx
        # PSUM tags, and if the scheduler hoists it early it parks in the
        # 4-deep PE wait queue and blocks the scores ACT is waiting on.
        # ctx for heads (2*half, 2*half+1) packs into one [128, TB] bank;
        # the softmax denominators accumulate separately on den row h via
        # one-hot lhsT columns.
        first = pair == 0
        last = pair == NKC // 2 - 1
        for half in range(2):
            for j in range(2):
                h = 2 * half + j
                mm = nc.tensor.matmul(
                    out=st["ctx"][h],
                    lhsT=va[:, 2 * pair:2 * pair + 2, h, 0:HD + 1],
                    rhs=pr[half][:, :, j * TB:(j + 1) * TB],
                    start=first, stop=last,
                    perf_mode=DRM,
                )
                if after is not None:
                    mm.ins.add_dependency(after.ins.name, _DEP_SYNC)

    def b_new_state(qb):
        return {
            "ctx": [
                ps_ctx.tile([HD + 1, TB], FP, tag=f"cx{h}", name=f"ctx{qb}_{h}")
                for h in range(FH)
            ],
        }

    def b_finish(qb, st):
        """normalize ctx, compute out-projection partials for ALL output
        columns from the core's own 256 ctx rows, ReduceScatter-add them.

        Norm is 1 reciprocal + per-chunk (bcast matmul, copy, mul): the
        bcast/copy run on PE + DVE/Pool which are idle at qb boundaries, so
        ACT never stalls behind this chain."""
        ctxT = pB_c2sb.tile([64, FH, TB], F8, tag="c2sb", name=f"ctxT{qb}")
        rdens = []
        for h in range(FH):
            rden = pB_rows.tile([1, TB], FR, tag=f"rden{h % 2}",
                                name=f"rden{qb}_{h}")
            nc.vector.reciprocal(out=rden, in_=st["ctx"][h][HD:HD + 1, :])
            rdens.append(rden)
        for h in range(FH):
            bcp = ps_sc.tile([64, TB], FP, tag="sc", name=f"bcp{qb}_{h}")
            nc.tensor.matmul(out=bcp, lhsT=ones_row[0:1, 0:64], rhs=rdens[h],
                             start=True, stop=True)
            bcs = pB_bcs.tile([64, TB], FP, tag=f"bcs{h % 2}", name=f"bcs{qb}_{h}")
            nc.vector.tensor_copy(out=bcs, in_=bcp)
            nc.vector.tensor_mul(
                out=ctxT[:, h, :], in0=st["ctx"][h][0:HD, :], in1=bcs)
        # partial out-projection: K=256 own ctx features as one DoubleRow
        # per 128-column chunk; SC_OUT folds into the fp16 evacuation
        poev = pB_poev.tile([128, DC, TB], F16, tag="poev", name=f"poev{qb}")
        po_tags = ["cx0", "cx1", "cx2", "cx3"]
        for m in range(DC):
            po = ps_ctx.tile([128, TB], FP, tag=po_tags[m % 4],
                             name=f"po{qb}_{m}")
            for c in range(2):
                nc.tensor.matmul(
                    out=po,
                    lhsT=woutr_sb[:, 2 * c:2 * c + 2, m * 128:(m + 1) * 128],
                    rhs=ctxT[:, 2 * c:2 * c + 2, :],
                    start=(c == 0), stop=(c == 1), perf_mode=DRM,
                )
            if qb == NB - 1 and m % 2 == 0:
                # last qb: ACT is idle after the final exp - split the evacs
                # across ACT and DVE so they drain in parallel
                nc.scalar.activation(out=poev[:, m, :], in_=po, func=AF.Copy,
                                     scale=SC_OUT)
            else:
                nc.vector.tensor_scalar_mul(out=poev[:, m, :], in0=po,
                                            scalar1=SC_OUT)
        rs_in = dramp.tile([D, TB], F16, tag="rsin", name=f"rsin{qb}")
        rs_in_r = rs_in.rearrange("(m p) tk -> p m tk", p=128)
        nc.sync.dma_start(out=rs_in_r[:, 0:DC // 2, :], in_=poev[:, 0:DC // 2, :])
        nc.sync.dma_start(out=rs_in_r[:, DC // 2:, :], in_=poev[:, DC // 2:, :])
        rs_out = dramp.tile([FQ, TB], F16, tag="rsout", name=f"rsout{qb}")
        nc.gpsimd.collective_compute(
            "ReduceScatter",
            mybir.AluOpType.add,
            replica_groups=GROUPS,
            ins=[rs_in.opt()],
            outs=[rs_out.opt()],
        )
        return rs_out

    sts = {}
    calls = {}
    outT_r = t["outT"].ap().rearrange("(m p) tk -> p m tk", p=128)

    def phase_c(qb, rs_out):
        """post-collective: gate + residual + store. No matmuls left here."""
        qs = slice(qb * TB, (qb + 1) * TB)
        xres_sb = pC_xr.tile([128, 2, TB], FP, tag="xres_sb", name=f"xres{qb}")
        nc.sync.dma_start(out=xres_sb, in_=xres_r[:, :, qs])
        rssb = pC_rs.tile([128, 2, TB], F16, tag="rssb", name=f"rssb{qb}")
        nc.sync.dma_start(
            out=rssb, in_=rs_out.rearrange("(m p) tk -> p m tk", p=128)
        )
        osb = pC_osb.tile([128, 2, TB], FP, tag="osb", name=f"osb{qb}")
        for m in range(2):
            # m=0 on DVE, m=1 on Pool (all-SBUF ops): the two chunks run in
            # parallel and each store overlaps the other chunk's math
            eng = nc.vector if m == 0 else nc.gpsimd
            eng.tensor_mul(
                out=osb[:, m, :], in0=rssb[:, m, :], in1=gT[:, m, qs])
            eng.tensor_add(
                out=osb[:, m, :], in0=osb[:, m, :], in1=xres_sb[:, m, :])
            nc.sync.dma_start(out=outT_r[:, m:m + 1, qs], in_=osb[:, m:m + 1, :])

    def phase_b(qb, fin_prev=None, c_prev=None):
        """full B for qb >= 1. AV runs three pairs behind scores;
        b_finish(qb-1) is deferred until two pairs of this qb's scores are
        emitted (its bcp broadcasts otherwise hold the sc rotation and
        starve ACT at the boundary - ReduceScatter has ample slack to
        absorb the later start); C(qb-2) lands at pair 5, when its
        ReduceScatter result has already arrived."""
        st = b_new_state(qb)
        sts[qb] = st
        prs = {}
        for pair in range(NKC // 2):
            pr01 = pB_pr.tile([128, 2, 2 * TB], F8, tag="pr", name=f"pr{qb}_{pair}_0")
            pr23 = pB_pr.tile([128, 2, 2 * TB], F8, tag="pr", name=f"pr{qb}_{pair}_1")
            smm = b_scores_pair(qb, pair, (pr01, pr23))
            prs[pair] = (pr01, pr23)
            if pair == 1 and fin_prev is not None:
                calls[fin_prev] = b_finish(fin_prev, sts.pop(fin_prev))
            if pair >= 3:
                b_av_pair(pair - 3, prs.pop(pair - 3), st, after=smm)
            if pair == 5 and c_prev is not None:
                phase_c(c_prev, calls[c_prev])
        for pair in sorted(prs):
            b_av_pair(pair, prs.pop(pair), st)

    # ------------------- emission schedule --------------------------------
    # B0's scores+exp are interleaved right after the A1 block that produces
    # their k tiles (probs parked in pB_pr0; AV deferred until PSUM ctx tags
    # are free after A), so ACT is busy from the first block on.
    pr0 = {}

    def b0_chunk(i):
        for pair in range(2 * i, 2 * i + 2):
            pr01 = pB_pr0.tile([128, 2, 2 * TB], F8, tag="pr0", name=f"pr0_{pair}_0")
            pr23 = pB_pr0.tile([128, 2, 2 * TB], F8, tag="pr0", name=f"pr0_{pair}_1")
            b_scores_pair(0, pair, (pr01, pr23))
            pr0[pair] = (pr01, pr23)

    load_x(0)
    load_weights()
    load_x(1)
    load_x(2)
    load_x(3)
    # all four stats blocks first (x is resident; they only chain through
    # one PSUM row tag each), then projections+B0 scores stream behind them
    phase_a0(0)
    phase_a0(1)
    phase_a0(2)
    phase_a0(3)
    phase_a1_qkg(0)
    b0_chunk(0)
    phase_a1_v(0)
    phase_a1_qkg(1)
    b0_chunk(1)
    phase_a1_v(1)
    phase_a1_qkg(2)
    b0_chunk(2)
    phase_a1_v(2)
    phase_a1_qkg(3)
    b0_chunk(3)
    phase_a1_v(3)
    st0 = b_new_state(0)
    sts[0] = st0
    for pair in range(NKC // 2):
        b_av_pair(pair, pr0.pop(pair), st0)
    phase_b(1, fin_prev=0)
    phase_b(2, fin_prev=1, c_prev=0)
    phase_b(3, fin_prev=2, c_prev=1)
    calls[3] = b_finish(3, sts.pop(3))
    phase_c(2, calls[2])
    phase_c(3, calls[3])

    stack.close()


def build_nc():
    if "nc" in _NC_CACHE:
        return _NC_CACHE["nc"]
    nc = bacc.Bacc("TRN2", target_bir_lowering=False, debug=False, num_devices=NCORES)
    t = {}
    t["xT"] = nc.dram_tensor("xT", [D, S], F8, kind="ExternalInput")
    t["xT16"] = nc.dram_tensor("xT16", [D, S], F16, kind="ExternalInput")
    t["xres"] = nc.dram_tensor("xres", [FQ, S], FP, kind="ExternalInput")
    t["wqkg"] = nc.dram_tensor("wqkg", [D, 3 * FQ], F8, kind="ExternalInput")
    t["wv"] = nc.dram_tensor("wv", [D, FQ], F8, kind="ExternalInput")
    t["wout"] = nc.dram_tensor("wout", [FQ, D], F8, kind="ExternalInput")
    t["ncs"] = nc.dram_tensor("ncs", [3 * FQ], FP, kind="ExternalInput")
    t["ncsv"] = nc.dram_tensor("ncsv", [FQ], FP, kind="ExternalInput")

    t["outT"] = nc.dram_tensor("outT", [FQ, S], FP, kind="ExternalOutput")
    with tile.TileContext(nc) as tc:
        _body(tc, t)
    nc.finalize()
    _NC_CACHE["nc"] = nc
    return nc


def _q8(a):
    return np.clip(np.asarray(a, np.float32), -240.0, 240.0).astype(
        ml_dtypes.float8_e4m3
    )


def make_in_maps(x, gamma, beta, w_qkv, b_qkv, w_out, b_out, w_gate, b_gate):
    x = np.asarray(x, np.float32)
    gamma = np.asarray(gamma, np.float32)
    beta = np.asarray(beta, np.float32)
    w_qkv = np.asarray(w_qkv, np.float32)
    b_qkv = np.asarray(b_qkv, np.float32)
    w_out = np.asarray(w_out, np.float32)
    b_out = np.asarray(b_out, np.float32)
    w_gate = np.asarray(w_gate, np.float32)
    b_gate = np.asarray(b_gate, np.float32)

    assert np.all(b_qkv == 0) and np.all(b_out == 0) and np.all(b_gate == 0) \
        and np.all(beta == 0), "kernel build assumes zero biases"

    xT = [np.ascontiguousarray(x[b].T) for b in range(B)]

    xT8 = [_q8(xb) for xb in xT]
    in_maps = []
    for c in range(NCORES):
        b, g = divmod(c, 4)
        cols = slice(g * FQ, (g + 1) * FQ)
        wq = w_qkv[:, 0 * D:1 * D][:, cols]
        wk = w_qkv[:, 1 * D:2 * D][:, cols]
        wv = w_qkv[:, 2 * D:3 * D][:, cols]
        wg = w_gate[:, cols]

        gfold = lambda w: gamma[:, None] * w

        wq8 = _q8(gfold(wq) * SW_QK)
        wk8 = _q8(gfold(wk) * SW_QK)
        wg8 = _q8(gfold(wg) * SW_G)
        wv8 = _q8(gfold(wv) * SW_V)
        wout8 = _q8(w_out[cols, :] * SW_O)

        wqkg8 = np.ascontiguousarray(np.concatenate(
            [wq8, wk8, wg8], axis=1, dtype=ml_dtypes.float8_e4m3))
        # mean-correction column sums over the *quantized* weights (exact)
        ncs = -np.concatenate(
            [wq8, wk8, wg8], axis=1).astype(np.float32).sum(axis=0)
        ncsv = -wv8.astype(np.float32).sum(axis=0)

        in_maps.append({
            "xT": xT8[b],
            "xT16": xT[b].astype(np.float16),
            "xres": np.ascontiguousarray(xT[b][cols, :]),
            "wqkg": wqkg8,
            "ncs": ncs.astype(np.float32),
            "ncsv": ncsv.astype(np.float32),
            "wv": np.ascontiguousarray(wv8),
            "wout": np.ascontiguousarray(wout8),
        })
    return in_maps


def run_device(in_maps):
    nc = build_nc()
    return run_bass_kernel_spmd(nc, in_maps, list(range(NCORES)))


def assemble(results):
    out = np.empty((B, S, D), np.float32)
    for c in range(NCORES):
        b, g = divmod(c, 4)
        out[b][:, g * FQ:(g + 1) * FQ] = results[c]["outT"].T
    return out


def kernel(**inputs):
    in_maps = make_in_maps(**inputs)
    res = run_device(in_maps)
    return assemble(res.results)
